# revision 6
# baseline (speedup 1.0000x reference)
"""Tree-GRU (arity-8, depth-5) over embedded leaves on 8 TRN2 NeuronCores.

Sharding: data-parallel over subtrees. Each core takes 4096 contiguous leaves
and runs levels 5..2 of the tree locally (512 -> 64 -> 8 -> 1 parents). The
root (level 1, 8 children = the 8 cores' level-2 outputs) is a trivial
16-matvec GRU done on host after gathering the per-core [384] outputs.

Device layout is feature-transposed: tensors live as [128 part, 3 ktile, ...]
with feature f = 128*k + p, so the GRU matmuls contract the partition dim.
Node storage is flat leaf-order (child-fastest), so all elementwise state
updates and the level-boundary x_next writes are contiguous; only the matmul
rhs / gi reads use stride-8 child slices. Weights are host-pre-transposed
into lhsT tiles; matmul dtype bf16 with fp32 PSUM accumulation.

Leaf level: embedding gather (indirect DMA, bf16 table, 4 SWDGE queues)
feeds PE transposes; step 0 (h=0, gi-only) runs in 4 sub-chunks of 128
parents that track gather completion; steps 1-7 ping-pong 2 chunks of 256.

Small levels (64/8/1 parents): the input transform gi for the whole level
(all 8 children x all parents) is precomputed in one batched matmul pass
(biases folded in via a K=1 ones-column matmul) and stored in SBUF, so the
sequential per-step work is only the recurrent matmul + gate chain. Per step
the r/z gi rows are injected into PSUM via an identity matmul (one start=True
covering MM per bank), the hn bias via a K=3 one-hot matmul, and the h update
uses h' = zc*n + z*h with zc = sigmoid(-pre_z) so z*h is computed off-path
(gpsimd) while tanh runs. The 1/8 output-mean scale is folded into a
pre-scaled copy of W_ih used by the gi passes; per-step output sums are
accumulated (hacc) and the final step writes the next level's input directly
as a fused raw-sum add.
"""

import numpy as np
import ml_dtypes

ARITY = 8
DIM = 384
VOCAB = 32000
NCORES = 8
P = 128
J = 3  # DIM // 128 feature tiles
N_LEAVES = 32768
LEAVES_CORE = N_LEAVES // NCORES  # 4096

BF16 = ml_dtypes.bfloat16

_PROG_CACHE = {}


def _levels_for(n_leaves_core):
    levels = []
    p = n_leaves_core // ARITY
    while p >= 1:
        levels.append(p)
        p //= ARITY
    assert levels[-1] == 1
    return levels


def _emit(tc, nc, aps, n_leaves_core):
    import concourse.mybir as mybir
    import concourse.bass as bass
    from concourse.masks import make_identity

    f32 = mybir.dt.float32
    bf16 = mybir.dt.bfloat16
    Sig = mybir.ActivationFunctionType.Sigmoid
    Tanh = mybir.ActivationFunctionType.Tanh
    Add = mybir.AluOpType.add
    Sub = mybir.AluOpType.subtract
    Mult = mybir.AluOpType.mult

    (tokens, embed, wih_t, wih_s, whh_t, biases, biases_mm, bias1, onehot3,
     ones, out_xh) = aps
    levels = _levels_for(n_leaves_core)
    P5 = levels[0]
    n_gtiles = n_leaves_core // P  # 32

    from contextlib import ExitStack

    with ExitStack() as ctx:
        const = ctx.enter_context(tc.tile_pool(name="const", bufs=1))
        xpool = ctx.enter_context(tc.tile_pool(name="xpool", bufs=1))
        state = ctx.enter_context(tc.tile_pool(name="state", bufs=1))
        gates = ctx.enter_context(tc.tile_pool(name="gates", bufs=4))
        gpool = ctx.enter_context(tc.tile_pool(name="gpool", bufs=1))
        pspool = ctx.enter_context(tc.tile_pool(name="pspool", bufs=8, space="PSUM"))

        # ---- tokens first, then kick off all gathers (4 SWDGE queues) ----
        tok_sb = const.tile([P, n_gtiles], mybir.dt.int32)
        nc.sync.dma_start(tok_sb[:], tokens.rearrange("(g p) -> p g", p=P))
        ident = const.tile([P, P], bf16)
        make_identity(nc, ident[:])

        xgs = []
        for g in range(n_gtiles):
            xg = gpool.tile([P, DIM], bf16, name="xg", tag="xg", bufs=n_gtiles)
            gi_inst = nc.gpsimd.indirect_dma_start(
                out=xg[:],
                out_offset=None,
                in_=embed[:],
                in_offset=bass.IndirectOffsetOnAxis(ap=tok_sb[:, g : g + 1], axis=0),
            )
            if g % 4 != 0:
                gi_inst.ins.queue = f"qPoolDynamic{g % 4}"
            xgs.append(xg)

        # ---- constants / weights ----
        wih_sb = const.tile([P, J, 9, P], bf16)
        wih_s_sb = const.tile([P, J, 9, P], bf16)
        whh_sb = const.tile([P, J, 9, P], bf16)
        bias_sb = const.tile([P, 12], f32)
        bias3_sb = const.tile([3, 4, P], bf16)
        bias1_sb = const.tile([1, 9, P], bf16)
        onehot3_sb = const.tile([3, 3, 512], bf16)
        ones_sb = const.tile([1, 512], bf16)
        nc.sync.dma_start(wih_sb[:], wih_t[:])
        nc.sync.dma_start(wih_s_sb[:], wih_s[:])
        nc.sync.dma_start(whh_sb[:], whh_t[:])
        nc.sync.dma_start(bias_sb[:], biases[:])
        nc.sync.dma_start(bias3_sb[:], biases_mm[:])
        nc.sync.dma_start(bias1_sb[:], bias1[:])
        nc.sync.dma_start(onehot3_sb[:], onehot3[:])
        nc.sync.dma_start(ones_sb[:], ones[:])

        # flat leaf-order x per level: [P, J, 8*Pl], child-fastest
        x_in = {}
        for Pl in levels:
            x_in[Pl] = xpool.tile([P, J, ARITY * Pl], bf16, name=f"x{Pl}", tag=f"x{Pl}")

        def xcv(Pl):
            # child-sliced view [P, J, Pl, 8]
            return x_in[Pl][:].rearrange("p j (q c) -> p j q c", c=ARITY)

        def psum_tile():
            return pspool.tile([P, 512], f32, name="ps", tag="ps")

        def new_state(name, dtype, Pl):
            return state.tile([P, J, Pl], dtype, name=f"{name}{Pl}", tag=f"{name}{Pl}")

        # =====================  LEAF LEVEL (Pl = P5)  =====================
        x5 = x_in[P5]
        h = new_state("h", bf16, P5)
        hacc = new_state("hacc", f32, P5)

        def emit_transposes(g0, g1):
            for g in range(g0, g1):
                for j in range(J):
                    tp = pspool.tile([P, 512], bf16, name="tp", tag="ps")
                    nc.tensor.transpose(
                        tp[:, :P], xgs[g][:, j * P : (j + 1) * P], ident[:]
                    )
                    nc.vector.tensor_copy(
                        out=x5[:, j, P * g : P * (g + 1)], in_=tp[:, :P]
                    )

        level_csum = [None]

        with nc.named_scope("leaf_t0"):
            # step 0: h=0, gi only; 4 sub-chunks of 128 parents (8 gtiles each)
            NSC = P5 // 4  # 128 parents per sub-chunk
            gsc = n_gtiles // 4
            for sc in range(4):
                emit_transposes(sc * gsc, (sc + 1) * gsc)
                sl = slice(sc * NSC, (sc + 1) * NSC)
                c0 = ARITY - 1  # first GRU input is the last child
                ps_r = [psum_tile()[:, :NSC] for _ in range(3)]
                ps_z = [psum_tile()[:, :NSC] for _ in range(3)]
                ps_in = [psum_tile()[:, :NSC] for _ in range(3)]
                for ps, moff in ((ps_r, 0), (ps_z, 3), (ps_in, 6)):
                    for m in range(3):
                        for k in range(J):
                            nc.tensor.matmul(
                                ps[m],
                                wih_sb[:, k, moff + m, :],
                                xcv(P5)[:, k, sl, c0],
                                start=(k == 0),
                                stop=(k == 2),
                            )
                r_sb = gates.tile([P, J, NSC], bf16, name="r0", tag="r0")
                z_sb = gates.tile([P, J, NSC], bf16, name="z0", tag="z0")
                n_sb = gates.tile([P, J, NSC], bf16, name="n0", tag="n0")
                rhn = gates.tile([P, J, NSC], f32, name="rhn0", tag="rhn0")
                t1 = gates.tile([P, J, NSC], bf16, name="t10", tag="t10")
                for m in range(3):
                    nc.scalar.activation(
                        r_sb[:, m], ps_r[m], Sig, bias=bias_sb[:, m : m + 1]
                    )
                for m in range(3):
                    nc.scalar.activation(
                        z_sb[:, m], ps_z[m], Sig, bias=bias_sb[:, 3 + m : 4 + m]
                    )
                for m in range(3):
                    nc.vector.tensor_scalar_mul(
                        rhn[:, m], r_sb[:, m], bias_sb[:, 6 + m : 7 + m]
                    )
                for m in range(3):
                    nc.vector.tensor_tensor(
                        out=rhn[:, m], in0=rhn[:, m], in1=ps_in[m], op=Add
                    )
                for m in range(3):
                    nc.scalar.activation(
                        n_sb[:, m], rhn[:, m], Tanh, bias=bias_sb[:, 9 + m : 10 + m]
                    )
                # h0=0: h' = n - z*n
                hsl = h[:, :, sl]
                nc.vector.tensor_tensor(out=t1[:], in0=z_sb[:], in1=n_sb[:], op=Mult)
                nc.vector.tensor_tensor(out=hsl, in0=n_sb[:], in1=t1[:], op=Sub)
                nc.gpsimd.tensor_copy(out=hacc[:, :, sl], in_=hsl)

        # steps 1..7: 2 chunks of 256
        NCH = 256
        nch = P5 // NCH
        for t in range(1, ARITY):
            c = ARITY - 1 - t
            with nc.named_scope(f"leaf_t{t}"):
                for ch in range(nch):
                    sl = slice(ch * NCH, (ch + 1) * NCH)
                    ps_r = [psum_tile()[:, :NCH] for _ in range(3)]
                    ps_z = [psum_tile()[:, :NCH] for _ in range(3)]
                    ps_in = [psum_tile()[:, :NCH] for _ in range(3)]
                    ps_hn = [psum_tile()[:, :NCH] for _ in range(3)]
                    for ps, moff in ((ps_r, 0), (ps_z, 3), (ps_in, 6)):
                        for m in range(3):
                            for k in range(J):
                                nc.tensor.matmul(
                                    ps[m],
                                    wih_sb[:, k, moff + m, :],
                                    xcv(P5)[:, k, sl, c],
                                    start=(k == 0),
                                    stop=(k == 2 and moff == 6),
                                )
                    for ps, moff in ((ps_r, 0), (ps_z, 3), (ps_hn, 6)):
                        for m in range(3):
                            for k in range(J):
                                nc.tensor.matmul(
                                    ps[m],
                                    whh_sb[:, k, moff + m, :],
                                    h[:, k, sl],
                                    start=(k == 0 and moff == 6),
                                    stop=(k == 2),
                                )

                    r_sb = gates.tile([P, J, NCH], bf16, name="r_sb", tag="r_sb")
                    z_sb = gates.tile([P, J, NCH], bf16, name="z_sb", tag="z_sb")
                    n_sb = gates.tile([P, J, NCH], bf16, name="n_sb", tag="n_sb")
                    rhn = gates.tile([P, J, NCH], f32, name="rhn", tag="rhn")
                    t1 = gates.tile([P, J, NCH], bf16, name="t1", tag="t1")

                    for m in range(3):
                        nc.scalar.activation(
                            r_sb[:, m], ps_r[m], Sig, bias=bias_sb[:, m : m + 1]
                        )
                    for m in range(3):
                        nc.scalar.activation(
                            z_sb[:, m], ps_z[m], Sig, bias=bias_sb[:, 3 + m : 4 + m]
                        )
                    for m in range(3):
                        nc.vector.scalar_tensor_tensor(
                            out=rhn[:, m],
                            in0=ps_hn[m],
                            scalar=bias_sb[:, 6 + m : 7 + m],
                            in1=r_sb[:, m],
                            op0=Add,
                            op1=Mult,
                        )
                    for m in range(3):
                        nc.vector.tensor_tensor(
                            out=rhn[:, m], in0=rhn[:, m], in1=ps_in[m], op=Add
                        )
                    for m in range(3):
                        nc.scalar.activation(
                            n_sb[:, m], rhn[:, m], Tanh, bias=bias_sb[:, 9 + m : 10 + m]
                        )

                    # h' = n + z*(h - n)
                    hsl = h[:, :, sl]
                    nc.vector.tensor_tensor(out=t1[:], in0=hsl, in1=n_sb[:], op=Sub)
                    nc.vector.tensor_tensor(out=t1[:], in0=z_sb[:], in1=t1[:], op=Mult)
                    nc.vector.tensor_tensor(out=hsl, in0=n_sb[:], in1=t1[:], op=Add)
                    if t == ARITY - 1:
                        qsl = slice(ch * NCH // ARITY, (ch + 1) * NCH // ARITY)
                        if ch == 0:
                            csum = state.tile(
                                [P, J, P5 // ARITY], f32, name="csum5", tag="csum5"
                            )
                            level_csum[0] = csum
                        nc.vector.tensor_reduce(
                            out=level_csum[0][:, :, qsl],
                            in_=hsl.rearrange("p j (q c) -> p j q c", c=ARITY),
                            axis=mybir.AxisListType.X,
                            op=Add,
                        )
                        xn = x_in[P5 // ARITY]
                        for j in range(J):
                            eng = nc.gpsimd if j == 2 else nc.vector
                            eng.tensor_tensor(
                                out=xn[:, j, ch * NCH : (ch + 1) * NCH],
                                in0=hacc[:, j, sl],
                                in1=hsl[:, j],
                                op=Add,
                            )
                    else:
                        nc.gpsimd.tensor_tensor(
                            out=hacc[:, :, sl], in0=hacc[:, :, sl], in1=hsl, op=Add
                        )

        # =====================  SMALL LEVELS (64, 8, 1)  ==================
        for Pl in levels[1:]:
            NC8 = ARITY * Pl  # children count = gi batch size
            with nc.named_scope(f"gi_{Pl}"):
                gi_sb = xpool.tile([P, 9, NC8], bf16, name=f"gi{Pl}", tag=f"gi{Pl}")
                # m-order: z (3,4,5) first so step 0's z-inject unblocks early,
                # then r (0,1,2), then n (6,7,8)
                for m in (3, 4, 5, 0, 1, 2, 6, 7, 8):
                    ps = psum_tile()[:, :NC8]
                    nc.tensor.matmul(
                        ps, bias1_sb[:, m, :], ones_sb[:, :NC8],
                        start=True, stop=False,
                    )
                    for k in range(J):
                        nc.tensor.matmul(
                            ps,
                            wih_s_sb[:, k, m, :],
                            x_in[Pl][:, k, :],
                            start=False,
                            stop=(k == 2),
                        )
                    nc.vector.tensor_copy(out=gi_sb[:, m, :], in_=ps)

            giv = gi_sb[:].rearrange("p m (q c) -> p m q c", c=ARITY)
            csum = level_csum[0]
            h = new_state("h", bf16, Pl)
            hacc = new_state("hacc", f32, Pl)
            nc.scalar.mul(h[:], csum[:], 1.0 / ARITY)

            for t in range(ARITY):
                c = ARITY - 1 - t
                with nc.named_scope(f"lv{Pl}_t{t}"):
                    N3 = 3 * Pl
                    ps_z, ps_r, ps_hn = psum_tile(), psum_tile(), psum_tile()

                    def view3(pst):
                        return pst[:, :N3].rearrange("p (j n) -> p j n", j=3)

                    def msl(pst, m):
                        return pst[:, m * Pl : (m + 1) * Pl]

                    # z first: inject gi_z (start=True), then hh
                    nc.tensor.matmul(
                        ps_z[:, :N3], ident[:], giv[:, 3:6, :, c],
                        start=True, stop=False,
                    )
                    for m in range(3):
                        for k in range(J):
                            nc.tensor.matmul(
                                msl(ps_z, m), whh_sb[:, k, 3 + m, :], h[:, k, :],
                                start=False, stop=(m == 2 and k == 2),
                            )
                    # r
                    nc.tensor.matmul(
                        ps_r[:, :N3], ident[:], giv[:, 0:3, :, c],
                        start=True, stop=False,
                    )
                    for m in range(3):
                        for k in range(J):
                            nc.tensor.matmul(
                                msl(ps_r, m), whh_sb[:, k, m, :], h[:, k, :],
                                start=False, stop=(m == 2 and k == 2),
                            )
                    # hn: bias via one-hot, then hh
                    nc.tensor.matmul(
                        ps_hn[:, :N3], bias3_sb[:, 2, :], onehot3_sb[:, :, :Pl],
                        start=True, stop=False,
                    )
                    for m in range(3):
                        for k in range(J):
                            nc.tensor.matmul(
                                msl(ps_hn, m), whh_sb[:, k, 6 + m, :], h[:, k, :],
                                start=False, stop=(m == 2 and k == 2),
                            )

                    z_sb = gates.tile([P, J, Pl], bf16, name="z_sb", tag="z_sb")
                    zc_sb = gates.tile([P, J, Pl], bf16, name="zc_sb", tag="zc_sb")
                    r_sb = gates.tile([P, J, Pl], bf16, name="r_sb", tag="r_sb")
                    n_sb = gates.tile([P, J, Pl], bf16, name="n_sb", tag="n_sb")
                    rhn = gates.tile([P, J, Pl], f32, name="rhn", tag="rhn")
                    t1 = gates.tile([P, J, Pl], f32, name="t1", tag="t1")
                    t2 = gates.tile([P, J, Pl], f32, name="t2", tag="t2")

                    nc.scalar.activation(z_sb[:], view3(ps_z), Sig)
                    nc.scalar.activation(zc_sb[:], view3(ps_z), Sig, scale=-1.0)
                    # t2 = z*h off-path while r/n compute
                    nc.gpsimd.tensor_tensor(out=t2[:], in0=z_sb[:], in1=h[:], op=Mult)
                    nc.scalar.activation(r_sb[:], view3(ps_r), Sig)
                    nc.vector.tensor_tensor(
                        out=rhn[:], in0=view3(ps_hn), in1=r_sb[:], op=Mult
                    )
                    nc.vector.tensor_tensor(
                        out=rhn[:], in0=rhn[:],
                        in1=giv[:, 6:9, :, c], op=Add,
                    )
                    nc.scalar.activation(n_sb[:], rhn[:], Tanh)
                    # h' = zc*n + z*h
                    nc.vector.tensor_tensor(out=t1[:], in0=zc_sb[:], in1=n_sb[:], op=Mult)
                    nc.vector.tensor_tensor(out=h[:], in0=t1[:], in1=t2[:], op=Add)

                    if t == 0:
                        nc.gpsimd.tensor_copy(out=hacc[:], in_=h[:])
                    elif t == ARITY - 1 and Pl > 1:
                        csum = state.tile(
                            [P, J, Pl // ARITY], f32, name=f"csum{Pl}", tag=f"csum{Pl}"
                        )
                        level_csum[0] = csum
                        nc.vector.tensor_reduce(
                            out=csum[:],
                            in_=h[:].rearrange("p j (q c) -> p j q c", c=ARITY),
                            axis=mybir.AxisListType.X,
                            op=Add,
                        )
                        xn = x_in[Pl // ARITY]
                        nc.vector.tensor_tensor(
                            out=xn[:], in0=hacc[:], in1=h[:], op=Add
                        )
                    else:
                        nc.gpsimd.tensor_tensor(
                            out=hacc[:], in0=hacc[:], in1=h[:], op=Add
                        )

        # ---- outputs: [P, 2, J] = (x_root, h_root) ----
        out_t = state.tile([P, 2, J], f32, name="out_t", tag="out_t")
        nc.scalar.mul(out_t[:, 0], hacc[:, :, 0], 1.0 / ARITY)
        nc.vector.tensor_copy(out=out_t[:, 1], in_=h[:, :, 0])
        nc.sync.dma_start(out_xh[:], out_t[:])


def _build_program(n_leaves_core):
    if n_leaves_core in _PROG_CACHE:
        return _PROG_CACHE[n_leaves_core]
    import concourse.bacc as bacc
    import concourse.mybir as mybir
    import concourse.tile as tile

    f32 = mybir.dt.float32
    bf16 = mybir.dt.bfloat16

    nc = bacc.Bacc(
        "TRN2",
        target_bir_lowering=False,
        debug=False,
        enable_asserts=False,
        num_devices=NCORES,
        num_swdge_queues=4,
    )
    tokens = nc.dram_tensor("tokens", [n_leaves_core], mybir.dt.int32, kind="ExternalInput").ap()
    embed = nc.dram_tensor("embed", [VOCAB, DIM], bf16, kind="ExternalInput").ap()
    wih_t = nc.dram_tensor("wih_t", [P, J, 9, P], bf16, kind="ExternalInput").ap()
    wih_s = nc.dram_tensor("wih_s", [P, J, 9, P], bf16, kind="ExternalInput").ap()
    whh_t = nc.dram_tensor("whh_t", [P, J, 9, P], bf16, kind="ExternalInput").ap()
    biases = nc.dram_tensor("biases", [P, 12], f32, kind="ExternalInput").ap()
    biases_mm = nc.dram_tensor("biases_mm", [3, 4, P], bf16, kind="ExternalInput").ap()
    bias1 = nc.dram_tensor("bias1", [1, 9, P], bf16, kind="ExternalInput").ap()
    onehot3 = nc.dram_tensor("onehot3", [3, 3, 512], bf16, kind="ExternalInput").ap()
    ones = nc.dram_tensor("ones", [1, 512], bf16, kind="ExternalInput").ap()
    out_xh = nc.dram_tensor("out_xh", [P, 2, J], f32, kind="ExternalOutput").ap()

    with tile.TileContext(nc) as tc:
        _emit(
            tc,
            nc,
            (tokens, embed, wih_t, wih_s, whh_t, biases, biases_mm, bias1,
             onehot3, ones, out_xh),
            n_leaves_core,
        )
    nc.compile()
    _PROG_CACHE[n_leaves_core] = nc
    return nc


def _retile_weights(w):
    # w: [1152, 384] -> lhsT tiles [128(k_part), 3(k), 9(m), 128(m_col)] bf16
    wt = np.ascontiguousarray(w.T)  # [384, 1152]
    wt = wt.reshape(J, P, 9, P).transpose(1, 0, 2, 3)
    return np.ascontiguousarray(wt).astype(BF16)


def _prep_bias(b_ih, b_hh):
    biases = np.zeros((P, 12), np.float32)
    comb = (b_ih + b_hh).reshape(9, P)
    biases[:, 0:6] = comb[0:6].T
    biases[:, 6:9] = b_hh.reshape(9, P)[6:9].T
    biases[:, 9:12] = b_ih.reshape(9, P)[6:9].T
    return biases


def _prep_bias_mm(b_ih, b_hh):
    # lhsT[k, ro, q] = bias[q, 3*ro + k]: the K=3 bias matmul against the
    # one-hot rhs yields out[q, (j, n)] = bias[q, 3*ro + j].
    b = _prep_bias(b_ih, b_hh)  # [128, 12] cols: r0..2 z0..2 hn0..2 in0..2
    out = b.T.reshape(4, 3, P).transpose(1, 0, 2)
    return np.ascontiguousarray(out).astype(BF16)


def _prep_bias1(b_ih, b_hh):
    # K=1 lhsT for the gi-precompute bias: out[col, :] += bias1[0, m, col].
    # r/z rows carry the combined input+hidden bias; n rows carry b_in only.
    out = np.zeros((1, 9, P), np.float32)
    comb = (b_ih + b_hh).reshape(9, P)
    out[0, 0:6] = comb[0:6]
    out[0, 6:9] = b_ih.reshape(9, P)[6:9]
    return out.astype(BF16)


def _prep_onehot3():
    out = np.zeros((3, 3, 512), np.float32)
    for k in range(3):
        out[k, k, :] = 1.0
    return out.astype(BF16)


def _gru_gates(x_t, h, w_ih, w_hh, b_ih, b_hh):
    gi = x_t @ w_ih.T + b_ih
    gh = h @ w_hh.T + b_hh
    i_r, i_z, i_n = np.split(gi, 3, axis=-1)
    h_r, h_z, h_n = np.split(gh, 3, axis=-1)
    r = 1.0 / (1.0 + np.exp(-(i_r + h_r)))
    z = 1.0 / (1.0 + np.exp(-(i_z + h_z)))
    n = np.tanh(i_n + r * h_n)
    return (1.0 - z) * n + z * h


def _root_gru(x_children, h0, w_ih, w_hh, b_ih, b_hh):
    h = h0.astype(np.float64)
    acc = np.zeros_like(h)
    for t in range(ARITY):
        x_t = x_children[ARITY - 1 - t].astype(np.float64)
        h = _gru_gates(x_t, h, w_ih.astype(np.float64), w_hh.astype(np.float64),
                       b_ih.astype(np.float64), b_hh.astype(np.float64))
        acc += h
    return (acc / ARITY).astype(np.float32)


def kernel(leaf_tokens, embed_table, w_ih, w_hh, b_ih, b_hh):
    from concourse.bass_utils import run_bass_kernel_spmd

    leaf_tokens = np.asarray(leaf_tokens, np.int32)
    embed_table = np.asarray(embed_table, np.float32)
    w_ih = np.asarray(w_ih, np.float32)
    w_hh = np.asarray(w_hh, np.float32)
    b_ih = np.asarray(b_ih, np.float32)
    b_hh = np.asarray(b_hh, np.float32)

    nc = _build_program(LEAVES_CORE)

    embed_bf = embed_table.astype(BF16)
    wih_t = _retile_weights(w_ih)
    wih_s = _retile_weights(w_ih / ARITY)
    whh_t = _retile_weights(w_hh)
    biases = _prep_bias(b_ih, b_hh)
    biases_mm = _prep_bias_mm(b_ih, b_hh)
    bias1 = _prep_bias1(b_ih, b_hh)
    ones = np.ones((1, 512), np.float32).astype(BF16)
    in_maps = []
    for core in range(NCORES):
        in_maps.append(
            {
                "tokens": np.ascontiguousarray(
                    leaf_tokens[core * LEAVES_CORE : (core + 1) * LEAVES_CORE]
                ),
                "embed": embed_bf,
                "wih_t": wih_t,
                "wih_s": wih_s,
                "whh_t": whh_t,
                "biases": biases,
                "biases_mm": biases_mm,
                "bias1": bias1,
                "onehot3": _prep_onehot3(),
                "ones": ones,
            }
        )
    res = run_bass_kernel_spmd(nc, in_maps, core_ids=list(range(NCORES)))

    xs = np.zeros((NCORES, DIM), np.float32)
    h8 = np.zeros((NCORES, DIM), np.float32)
    for core in range(NCORES):
        out = res.results[core]["out_xh"]  # [P, 2, J]
        xs[core] = out[:, 0].T.reshape(-1)
        h8[core] = out[:, 1].T.reshape(-1)

    h0 = h8.mean(axis=0)
    out = _root_gru(xs, h0, w_ih, w_hh, b_ih, b_hh)
    return out.reshape(1, 1, DIM)


# revision 10
# speedup vs baseline: 1.3642x; 1.3642x over previous
"""Tree-GRU (arity-8, depth-5) over embedded leaves on 8 TRN2 NeuronCores.

Sharding: data-parallel over subtrees. Each core takes 4096 contiguous leaves
and runs levels 5..2 of the tree locally (512 -> 64 -> 8 -> 1 parents). The
root (level 1, 8 children = the 8 cores' level-2 outputs) is a trivial
16-matvec GRU done on host after gathering the per-core [384] outputs.

Device layout is feature-transposed: tensors live as [128 part, 3 ktile, ...]
with feature f = 128*k + p, so the GRU matmuls contract the partition dim.
Node storage is flat leaf-order (child-fastest), so all elementwise state
updates and the level-boundary x_next writes are contiguous; only the matmul
rhs / gi reads use stride-8 child slices. Weights are host-pre-transposed
into lhsT tiles; matmul dtype bf16 with fp32 PSUM accumulation.

Leaf level: embedding gather (indirect DMA, bf16 table, 4 SWDGE queues)
feeds PE transposes; step 0 (h=0, gi-only) runs in 4 sub-chunks of 128
parents that track gather completion; steps 1-7 ping-pong 2 chunks of 256.

Small levels (64/8/1 parents): the input transform gi for the whole level
(all 8 children x all parents) is precomputed in one batched matmul pass
(biases folded in via a K=1 ones-column matmul) and stored in SBUF, so the
sequential per-step work is only the recurrent matmul + gate chain. Per step
the r/z gi rows are injected into PSUM via an identity matmul (one start=True
covering MM per bank), the hn bias via a K=3 one-hot matmul, and the h update
uses h' = zc*n + z*h with zc = sigmoid(-pre_z) so z*h is computed off-path
(gpsimd) while tanh runs. The 1/8 output-mean scale is folded into a
pre-scaled copy of W_ih used by the gi passes; per-step output sums are
accumulated (hacc) and the final step writes the next level's input directly
as a fused raw-sum add.
"""

import numpy as np
import ml_dtypes

ARITY = 8
DIM = 384
VOCAB = 32000
NCORES = 8
P = 128
J = 3  # DIM // 128 feature tiles
N_LEAVES = 32768
LEAVES_CORE = N_LEAVES // NCORES  # 4096

BF16 = ml_dtypes.bfloat16

_PROG_CACHE = {}


def _levels_for(n_leaves_core):
    levels = []
    p = n_leaves_core // ARITY
    while p >= 1:
        levels.append(p)
        p //= ARITY
    assert levels[-1] == 1
    return levels


def _emit(tc, nc, aps, n_leaves_core):
    import concourse.mybir as mybir
    import concourse.bass as bass
    from concourse.masks import make_identity

    f32 = mybir.dt.float32
    bf16 = mybir.dt.bfloat16
    Sig = mybir.ActivationFunctionType.Sigmoid
    Tanh = mybir.ActivationFunctionType.Tanh
    Add = mybir.AluOpType.add
    Sub = mybir.AluOpType.subtract
    Mult = mybir.AluOpType.mult

    (tokens, embed, wih_t, wih_s, whh_t, biases, biases_mm, bias1, onehot3,
     ones, out_xh) = aps
    levels = _levels_for(n_leaves_core)
    P5 = levels[0]
    n_gtiles = n_leaves_core // P  # 32

    from contextlib import ExitStack

    with ExitStack() as ctx:
        const = ctx.enter_context(tc.tile_pool(name="const", bufs=1))
        xpool = ctx.enter_context(tc.tile_pool(name="xpool", bufs=1))
        state = ctx.enter_context(tc.tile_pool(name="state", bufs=1))
        gates = ctx.enter_context(tc.tile_pool(name="gates", bufs=4))
        gpool = ctx.enter_context(tc.tile_pool(name="gpool", bufs=1))
        pspool = ctx.enter_context(tc.tile_pool(name="pspool", bufs=8, space="PSUM"))

        # ---- tokens first, then kick off all gathers (4 SWDGE queues) ----
        tok_sb = const.tile([P, n_gtiles], mybir.dt.int32)
        nc.sync.dma_start(tok_sb[:], tokens.rearrange("(g p) -> p g", p=P))
        ident = const.tile([P, P], bf16)
        make_identity(nc, ident[:])

        xgs = []
        for g in range(n_gtiles):
            xg = gpool.tile([P, DIM], bf16, name="xg", tag="xg", bufs=n_gtiles)
            gi_inst = nc.gpsimd.indirect_dma_start(
                out=xg[:],
                out_offset=None,
                in_=embed[:],
                in_offset=bass.IndirectOffsetOnAxis(ap=tok_sb[:, g : g + 1], axis=0),
            )
            if g % 4 != 0:
                gi_inst.ins.queue = f"qPoolDynamic{g % 4}"
            xgs.append(xg)

        # ---- constants / weights ----
        wih_sb = const.tile([P, J, 9, P], bf16)
        wih_s_sb = const.tile([P, J, 9, P], bf16)
        whh_sb = const.tile([P, J, 9, P], bf16)
        bias_sb = const.tile([P, 12], f32)
        bias3_sb = const.tile([3, 4, P], bf16)
        bias1_sb = const.tile([1, 9, P], bf16)
        onehot3_sb = const.tile([3, 3, 512], bf16)
        ones_sb = const.tile([1, 512], bf16)
        nc.sync.dma_start(wih_sb[:], wih_t[:])
        nc.sync.dma_start(wih_s_sb[:], wih_s[:])
        nc.sync.dma_start(whh_sb[:], whh_t[:])
        nc.sync.dma_start(bias_sb[:], biases[:])
        nc.sync.dma_start(bias3_sb[:], biases_mm[:])
        nc.sync.dma_start(bias1_sb[:], bias1[:])
        nc.sync.dma_start(onehot3_sb[:], onehot3[:])
        nc.sync.dma_start(ones_sb[:], ones[:])

        # child-major x per level: [P, J, 8, Pl] (contiguous matmul rhs)
        x_in = {}
        for Pl in levels:
            x_in[Pl] = xpool.tile([P, J, ARITY, Pl], bf16, name=f"x{Pl}", tag=f"x{Pl}")

        def psum_tile():
            return pspool.tile([P, 512], f32, name="ps", tag="ps")

        def new_state(name, dtype, Pl):
            return state.tile([P, J, Pl], dtype, name=f"{name}{Pl}", tag=f"{name}{Pl}")

        # =====================  LEAF LEVEL (Pl = P5)  =====================
        x5 = x_in[P5]
        h = new_state("h", bf16, P5)
        # permuted (child-major) accumulator so the level-end x_next add is a
        # contiguous write into the next level's child-major x
        hacc = state.tile([P, J, ARITY, P5 // ARITY], f32, name="hacc5", tag="hacc5")

        def emit_transposes(g0, g1):
            for g in range(g0, g1):
                for j in range(J):
                    tp = pspool.tile([P, 512], bf16, name="tp", tag="ps")
                    nc.tensor.transpose(
                        tp[:, :P], xgs[g][:, j * P : (j + 1) * P], ident[:]
                    )
                    nc.vector.tensor_copy(
                        out=x5[:, j, :, 16 * g : 16 * (g + 1)],
                        in_=tp[:, :P].rearrange("p (par c) -> p c par", c=ARITY),
                    )

        level_csum = [None]

        with nc.named_scope("leaf_t0"):
            # step 0: h=0, gi only; 4 sub-chunks of 128 parents (8 gtiles each)
            NSC = P5 // 4  # 128 parents per sub-chunk
            gsc = n_gtiles // 4
            c0 = ARITY - 1  # first GRU input is the last child
            for sc in range(4):
                emit_transposes(sc * gsc, (sc + 1) * gsc)
                sl = slice(sc * NSC, (sc + 1) * NSC)
                ps_r = [psum_tile()[:, :NSC] for _ in range(3)]
                ps_z = [psum_tile()[:, :NSC] for _ in range(3)]
                ps_in = [psum_tile()[:, :NSC] for _ in range(3)]
                for ps, moff in ((ps_r, 0), (ps_z, 3), (ps_in, 6)):
                    for m in range(3):
                        for k in range(J):
                            nc.tensor.matmul(
                                ps[m],
                                wih_sb[:, k, moff + m, :],
                                x5[:, k, c0, sl],
                                start=(k == 0),
                                stop=(k == 2),
                            )
                r_sb = gates.tile([P, J, NSC], bf16, name="r0", tag="r0")
                z_sb = gates.tile([P, J, NSC], bf16, name="z0", tag="z0")
                n_sb = gates.tile([P, J, NSC], bf16, name="n0", tag="n0")
                rhn = gates.tile([P, J, NSC], f32, name="rhn0", tag="rhn0")
                t1 = gates.tile([P, J, NSC], bf16, name="t10", tag="t10")
                for m in range(3):
                    nc.scalar.activation(
                        r_sb[:, m], ps_r[m], Sig, bias=bias_sb[:, m : m + 1]
                    )
                for m in range(3):
                    nc.scalar.activation(
                        z_sb[:, m], ps_z[m], Sig, bias=bias_sb[:, 3 + m : 4 + m]
                    )
                for m in range(3):
                    nc.vector.tensor_scalar_mul(
                        rhn[:, m], r_sb[:, m], bias_sb[:, 6 + m : 7 + m]
                    )
                for m in range(3):
                    nc.vector.tensor_tensor(
                        out=rhn[:, m], in0=rhn[:, m], in1=ps_in[m], op=Add
                    )
                for m in range(3):
                    nc.scalar.activation(
                        n_sb[:, m], rhn[:, m], Tanh, bias=bias_sb[:, 9 + m : 10 + m]
                    )
                # h0=0: h' = n - z*n
                hsl = h[:, :, sl]
                nc.vector.tensor_tensor(out=t1[:], in0=z_sb[:], in1=n_sb[:], op=Mult)
                nc.vector.tensor_tensor(out=hsl, in0=n_sb[:], in1=t1[:], op=Sub)
                nc.gpsimd.tensor_copy(
                    out=hacc[:, :, :, sc * (NSC // ARITY) : (sc + 1) * (NSC // ARITY)],
                    in_=hsl.rearrange("p j (q c) -> p j c q", c=ARITY),
                )

        NCH = 256
        nch = P5 // NCH
        for t in range(1, ARITY):
            c = ARITY - 1 - t
            with nc.named_scope(f"leaf_t{t}"):
                for ch in range(nch):
                    sl = slice(ch * NCH, (ch + 1) * NCH)
                    ps_r = [psum_tile()[:, :NCH] for _ in range(3)]
                    ps_z = [psum_tile()[:, :NCH] for _ in range(3)]
                    ps_in = [psum_tile()[:, :NCH] for _ in range(3)]
                    ps_hn = [psum_tile()[:, :NCH] for _ in range(3)]
                    for ps, moff in ((ps_r, 0), (ps_z, 3), (ps_in, 6)):
                        for m in range(3):
                            for k in range(J):
                                nc.tensor.matmul(
                                    ps[m],
                                    wih_sb[:, k, moff + m, :],
                                    x5[:, k, c, sl],
                                    start=(k == 0),
                                    stop=(k == 2 and moff == 6),
                                )
                    for ps, moff in ((ps_r, 0), (ps_z, 3), (ps_hn, 6)):
                        for m in range(3):
                            for k in range(J):
                                nc.tensor.matmul(
                                    ps[m],
                                    whh_sb[:, k, moff + m, :],
                                    h[:, k, sl],
                                    start=(k == 0 and moff == 6),
                                    stop=(k == 2),
                                )

                    r_sb = gates.tile([P, J, NCH], bf16, name="r_sb", tag="r_sb")
                    z_sb = gates.tile([P, J, NCH], bf16, name="z_sb", tag="z_sb")
                    n_sb = gates.tile([P, J, NCH], bf16, name="n_sb", tag="n_sb")
                    rhn = gates.tile([P, J, NCH], f32, name="rhn", tag="rhn")
                    t1 = gates.tile([P, J, NCH], bf16, name="t1", tag="t1")

                    for m in range(3):
                        nc.scalar.activation(
                            r_sb[:, m], ps_r[m], Sig, bias=bias_sb[:, m : m + 1]
                        )
                    for m in range(3):
                        nc.scalar.activation(
                            z_sb[:, m], ps_z[m], Sig, bias=bias_sb[:, 3 + m : 4 + m]
                        )
                    for m in range(3):
                        nc.vector.scalar_tensor_tensor(
                            out=rhn[:, m],
                            in0=ps_hn[m],
                            scalar=bias_sb[:, 6 + m : 7 + m],
                            in1=r_sb[:, m],
                            op0=Add,
                            op1=Mult,
                        )
                    for m in range(3):
                        nc.vector.tensor_tensor(
                            out=rhn[:, m], in0=rhn[:, m], in1=ps_in[m], op=Add
                        )
                    for m in range(3):
                        nc.scalar.activation(
                            n_sb[:, m], rhn[:, m], Tanh, bias=bias_sb[:, 9 + m : 10 + m]
                        )

                    # h' = n + z*(h - n)
                    hsl = h[:, :, sl]
                    nc.vector.tensor_tensor(out=t1[:], in0=hsl, in1=n_sb[:], op=Sub)
                    nc.vector.tensor_tensor(out=t1[:], in0=z_sb[:], in1=t1[:], op=Mult)
                    nc.vector.tensor_tensor(out=hsl, in0=n_sb[:], in1=t1[:], op=Add)
                    hperm = hsl.rearrange("p j (q c) -> p j c q", c=ARITY)
                    qsl = slice(ch * NCH // ARITY, (ch + 1) * NCH // ARITY)
                    if t == ARITY - 1:
                        if ch == 0:
                            csum = state.tile(
                                [P, J, P5 // ARITY], f32, name="csum5", tag="csum5"
                            )
                            level_csum[0] = csum
                        nc.vector.tensor_reduce(
                            out=level_csum[0][:, :, qsl],
                            in_=hsl.rearrange("p j (q c) -> p j q c", c=ARITY),
                            axis=mybir.AxisListType.X,
                            op=Add,
                        )
                        xn = x_in[P5 // ARITY]
                        for j in range(J):
                            eng = nc.gpsimd if j == 2 else nc.vector
                            eng.tensor_tensor(
                                out=xn[:, j, :, qsl],
                                in0=hacc[:, j, :, qsl],
                                in1=hperm[:, j],
                                op=Add,
                            )
                    else:
                        nc.gpsimd.tensor_tensor(
                            out=hacc[:, :, :, qsl],
                            in0=hacc[:, :, :, qsl],
                            in1=hperm,
                            op=Add,
                        )

        # =====================  SMALL LEVELS (64, 8, 1)  ==================
        for Pl in levels[1:]:
            NC8 = ARITY * Pl  # children count = gi batch size
            with nc.named_scope(f"gi_{Pl}"):
                # gi stored child-major [P, 9, 8, Pl]: the gi pass rhs is the
                # child-major x (contiguous), so PSUM comes out (c, q)-ordered
                gi_sb = xpool.tile([P, 9, ARITY, Pl], bf16, name=f"gi{Pl}",
                                   tag=f"gi{Pl}")
                # m-order: z (3,4,5) first so step 0's z-inject unblocks early,
                # then r (0,1,2), then n (6,7,8)
                for m in (3, 4, 5, 0, 1, 2, 6, 7, 8):
                    ps = psum_tile()[:, :NC8]
                    nc.tensor.matmul(
                        ps, bias1_sb[:, m, :], ones_sb[:, :NC8],
                        start=True, stop=False,
                    )
                    for k in range(J):
                        nc.tensor.matmul(
                            ps,
                            wih_s_sb[:, k, m, :],
                            x_in[Pl][:, k, :, :],
                            start=False,
                            stop=(k == 2),
                        )
                    nc.vector.tensor_copy(
                        out=gi_sb[:, m].rearrange("p c q -> p (c q)"), in_=ps
                    )

            csum = level_csum[0]
            h = new_state("h", bf16, Pl)
            hacc = new_state("hacc", f32, Pl)
            nc.scalar.mul(h[:], csum[:], 1.0 / ARITY)

            for t in range(ARITY):
                c = ARITY - 1 - t
                with nc.named_scope(f"lv{Pl}_t{t}"):
                    N3 = 3 * Pl
                    ps_z, ps_r, ps_hn = psum_tile(), psum_tile(), psum_tile()

                    def view3(pst):
                        return pst[:, :N3].rearrange("p (j n) -> p j n", j=3)

                    def msl(pst, m):
                        return pst[:, m * Pl : (m + 1) * Pl]

                    # z first: inject gi_z (start=True), then hh
                    nc.tensor.matmul(
                        ps_z[:, :N3], ident[:], gi_sb[:, 3:6, c, :],
                        start=True, stop=False,
                    )
                    for m in range(3):
                        for k in range(J):
                            nc.tensor.matmul(
                                msl(ps_z, m), whh_sb[:, k, 3 + m, :], h[:, k, :],
                                start=False, stop=(m == 2 and k == 2),
                            )
                    # r
                    nc.tensor.matmul(
                        ps_r[:, :N3], ident[:], gi_sb[:, 0:3, c, :],
                        start=True, stop=False,
                    )
                    for m in range(3):
                        for k in range(J):
                            nc.tensor.matmul(
                                msl(ps_r, m), whh_sb[:, k, m, :], h[:, k, :],
                                start=False, stop=(m == 2 and k == 2),
                            )
                    # hn: bias via one-hot, then hh
                    nc.tensor.matmul(
                        ps_hn[:, :N3], bias3_sb[:, 2, :], onehot3_sb[:, :, :Pl],
                        start=True, stop=False,
                    )
                    for m in range(3):
                        for k in range(J):
                            nc.tensor.matmul(
                                msl(ps_hn, m), whh_sb[:, k, 6 + m, :], h[:, k, :],
                                start=False, stop=(m == 2 and k == 2),
                            )

                    z_sb = gates.tile([P, J, Pl], bf16, name="z_sb", tag="z_sb")
                    zc_sb = gates.tile([P, J, Pl], bf16, name="zc_sb", tag="zc_sb")
                    r_sb = gates.tile([P, J, Pl], bf16, name="r_sb", tag="r_sb")
                    n_sb = gates.tile([P, J, Pl], bf16, name="n_sb", tag="n_sb")
                    rhn = gates.tile([P, J, Pl], f32, name="rhn", tag="rhn")
                    t1 = gates.tile([P, J, Pl], f32, name="t1", tag="t1")
                    t2 = gates.tile([P, J, Pl], f32, name="t2", tag="t2")

                    nc.scalar.activation(z_sb[:], view3(ps_z), Sig)
                    nc.scalar.activation(zc_sb[:], view3(ps_z), Sig, scale=-1.0)
                    # t2 = z*h off-path while r/n compute
                    nc.gpsimd.tensor_tensor(out=t2[:], in0=z_sb[:], in1=h[:], op=Mult)
                    nc.scalar.activation(r_sb[:], view3(ps_r), Sig)
                    nc.vector.tensor_tensor(
                        out=rhn[:], in0=view3(ps_hn), in1=r_sb[:], op=Mult
                    )
                    nc.vector.tensor_tensor(
                        out=rhn[:], in0=rhn[:], in1=gi_sb[:, 6:9, c, :], op=Add
                    )
                    nc.scalar.activation(n_sb[:], rhn[:], Tanh)
                    # h' = zc*n + z*h
                    nc.vector.tensor_tensor(out=t1[:], in0=zc_sb[:], in1=n_sb[:], op=Mult)
                    nc.vector.tensor_tensor(out=h[:], in0=t1[:], in1=t2[:], op=Add)

                    if t == 0:
                        nc.gpsimd.tensor_copy(out=hacc[:], in_=h[:])
                    elif t == ARITY - 1 and Pl > 1:
                        csum = state.tile(
                            [P, J, Pl // ARITY], f32, name=f"csum{Pl}", tag=f"csum{Pl}"
                        )
                        level_csum[0] = csum
                        nc.vector.tensor_reduce(
                            out=csum[:],
                            in_=h[:].rearrange("p j (q c) -> p j q c", c=ARITY),
                            axis=mybir.AxisListType.X,
                            op=Add,
                        )
                        xn = x_in[Pl // ARITY]
                        nc.vector.tensor_tensor(
                            out=xn[:],
                            in0=hacc[:].rearrange("p j (q c) -> p j c q", c=ARITY),
                            in1=h[:].rearrange("p j (q c) -> p j c q", c=ARITY),
                            op=Add,
                        )
                    else:
                        nc.gpsimd.tensor_tensor(
                            out=hacc[:], in0=hacc[:], in1=h[:], op=Add
                        )

        # ---- outputs: [P, 2, J] = (x_root, h_root) ----
        out_t = state.tile([P, 2, J], f32, name="out_t", tag="out_t")
        nc.scalar.mul(out_t[:, 0], hacc[:, :, 0], 1.0 / ARITY)
        nc.vector.tensor_copy(out=out_t[:, 1], in_=h[:, :, 0])
        nc.sync.dma_start(out_xh[:], out_t[:])


def _build_program(n_leaves_core):
    if n_leaves_core in _PROG_CACHE:
        return _PROG_CACHE[n_leaves_core]
    import concourse.bacc as bacc
    import concourse.mybir as mybir
    import concourse.tile as tile

    f32 = mybir.dt.float32
    bf16 = mybir.dt.bfloat16

    nc = bacc.Bacc(
        "TRN2",
        target_bir_lowering=False,
        debug=False,
        enable_asserts=False,
        num_devices=NCORES,
        num_swdge_queues=4,
    )
    tokens = nc.dram_tensor("tokens", [n_leaves_core], mybir.dt.int32, kind="ExternalInput").ap()
    embed = nc.dram_tensor("embed", [VOCAB, DIM], bf16, kind="ExternalInput").ap()
    wih_t = nc.dram_tensor("wih_t", [P, J, 9, P], bf16, kind="ExternalInput").ap()
    wih_s = nc.dram_tensor("wih_s", [P, J, 9, P], bf16, kind="ExternalInput").ap()
    whh_t = nc.dram_tensor("whh_t", [P, J, 9, P], bf16, kind="ExternalInput").ap()
    biases = nc.dram_tensor("biases", [P, 12], f32, kind="ExternalInput").ap()
    biases_mm = nc.dram_tensor("biases_mm", [3, 4, P], bf16, kind="ExternalInput").ap()
    bias1 = nc.dram_tensor("bias1", [1, 9, P], bf16, kind="ExternalInput").ap()
    onehot3 = nc.dram_tensor("onehot3", [3, 3, 512], bf16, kind="ExternalInput").ap()
    ones = nc.dram_tensor("ones", [1, 512], bf16, kind="ExternalInput").ap()
    out_xh = nc.dram_tensor("out_xh", [P, 2, J], f32, kind="ExternalOutput").ap()

    with tile.TileContext(nc) as tc:
        _emit(
            tc,
            nc,
            (tokens, embed, wih_t, wih_s, whh_t, biases, biases_mm, bias1,
             onehot3, ones, out_xh),
            n_leaves_core,
        )
    nc.compile()
    _PROG_CACHE[n_leaves_core] = nc
    return nc


def _retile_weights(w):
    # w: [1152, 384] -> lhsT tiles [128(k_part), 3(k), 9(m), 128(m_col)] bf16
    wt = np.ascontiguousarray(w.T)  # [384, 1152]
    wt = wt.reshape(J, P, 9, P).transpose(1, 0, 2, 3)
    return np.ascontiguousarray(wt).astype(BF16)


def _prep_bias(b_ih, b_hh):
    biases = np.zeros((P, 12), np.float32)
    comb = (b_ih + b_hh).reshape(9, P)
    biases[:, 0:6] = comb[0:6].T
    biases[:, 6:9] = b_hh.reshape(9, P)[6:9].T
    biases[:, 9:12] = b_ih.reshape(9, P)[6:9].T
    return biases


def _prep_bias_mm(b_ih, b_hh):
    # lhsT[k, ro, q] = bias[q, 3*ro + k]: the K=3 bias matmul against the
    # one-hot rhs yields out[q, (j, n)] = bias[q, 3*ro + j].
    b = _prep_bias(b_ih, b_hh)  # [128, 12] cols: r0..2 z0..2 hn0..2 in0..2
    out = b.T.reshape(4, 3, P).transpose(1, 0, 2)
    return np.ascontiguousarray(out).astype(BF16)


def _prep_bias1(b_ih, b_hh):
    # K=1 lhsT for the gi-precompute bias: out[col, :] += bias1[0, m, col].
    # r/z rows carry the combined input+hidden bias; n rows carry b_in only.
    out = np.zeros((1, 9, P), np.float32)
    comb = (b_ih + b_hh).reshape(9, P)
    out[0, 0:6] = comb[0:6]
    out[0, 6:9] = b_ih.reshape(9, P)[6:9]
    return out.astype(BF16)


def _prep_onehot3():
    out = np.zeros((3, 3, 512), np.float32)
    for k in range(3):
        out[k, k, :] = 1.0
    return out.astype(BF16)


def _gru_gates(x_t, h, w_ih, w_hh, b_ih, b_hh):
    gi = x_t @ w_ih.T + b_ih
    gh = h @ w_hh.T + b_hh
    i_r, i_z, i_n = np.split(gi, 3, axis=-1)
    h_r, h_z, h_n = np.split(gh, 3, axis=-1)
    r = 1.0 / (1.0 + np.exp(-(i_r + h_r)))
    z = 1.0 / (1.0 + np.exp(-(i_z + h_z)))
    n = np.tanh(i_n + r * h_n)
    return (1.0 - z) * n + z * h


def _root_gru(x_children, h0, w_ih, w_hh, b_ih, b_hh):
    h = h0.astype(np.float64)
    acc = np.zeros_like(h)
    for t in range(ARITY):
        x_t = x_children[ARITY - 1 - t].astype(np.float64)
        h = _gru_gates(x_t, h, w_ih.astype(np.float64), w_hh.astype(np.float64),
                       b_ih.astype(np.float64), b_hh.astype(np.float64))
        acc += h
    return (acc / ARITY).astype(np.float32)


def kernel(leaf_tokens, embed_table, w_ih, w_hh, b_ih, b_hh):
    from concourse.bass_utils import run_bass_kernel_spmd

    leaf_tokens = np.asarray(leaf_tokens, np.int32)
    embed_table = np.asarray(embed_table, np.float32)
    w_ih = np.asarray(w_ih, np.float32)
    w_hh = np.asarray(w_hh, np.float32)
    b_ih = np.asarray(b_ih, np.float32)
    b_hh = np.asarray(b_hh, np.float32)

    nc = _build_program(LEAVES_CORE)

    embed_bf = embed_table.astype(BF16)
    wih_t = _retile_weights(w_ih)
    wih_s = _retile_weights(w_ih / ARITY)
    whh_t = _retile_weights(w_hh)
    biases = _prep_bias(b_ih, b_hh)
    biases_mm = _prep_bias_mm(b_ih, b_hh)
    bias1 = _prep_bias1(b_ih, b_hh)
    ones = np.ones((1, 512), np.float32).astype(BF16)
    in_maps = []
    for core in range(NCORES):
        in_maps.append(
            {
                "tokens": np.ascontiguousarray(
                    leaf_tokens[core * LEAVES_CORE : (core + 1) * LEAVES_CORE]
                ),
                "embed": embed_bf,
                "wih_t": wih_t,
                "wih_s": wih_s,
                "whh_t": whh_t,
                "biases": biases,
                "biases_mm": biases_mm,
                "bias1": bias1,
                "onehot3": _prep_onehot3(),
                "ones": ones,
            }
        )
    res = run_bass_kernel_spmd(nc, in_maps, core_ids=list(range(NCORES)))

    xs = np.zeros((NCORES, DIM), np.float32)
    h8 = np.zeros((NCORES, DIM), np.float32)
    for core in range(NCORES):
        out = res.results[core]["out_xh"]  # [P, 2, J]
        xs[core] = out[:, 0].T.reshape(-1)
        h8[core] = out[:, 1].T.reshape(-1)

    h0 = h8.mean(axis=0)
    out = _root_gru(xs, h0, w_ih, w_hh, b_ih, b_hh)
    return out.reshape(1, 1, DIM)
